# revision 11
# baseline (speedup 1.0000x reference)
"""BAM-style attention block (avgpool8 -> 1024-token attention -> nearest-upsample + residual)
as a distributed Bass kernel on 8 TRN2 NeuronCores.

Sharding: core = b*2 + half  (b = batch 0..3, half = H-half 0..1).

Schedule (per core):
  stream:  64 chunks of [128ch, 8rows, 256] stream through one 16-slot SBUF
           pool (sync ring).  DVE pools each chunk to 32 tokens (f32 sums);
           the 1/64 pool scale is folded into the conv weights.  Pooled
           sums are staged to DRAM via gpsimd cast-DMA (f32->bf16, no DVE
           dependency on the trigger path) and exchanged pairwise with 5
           AllGathers: cg0..cg2 whole, cg3 in two halves (the CC stream
           has ~21us of per-op overhead, so the last exchange is kept
           small and the schedule keeps the stream clear for it).
           Remote halves recovered rank-agnostically as (h0+h1)-local on
           gpsimd; k/v-projection partials for remote cg0..2 and local
           cg0..2 accumulate into bf16 SBUF at stream positions where
           their data is provably ready, so the post-exchange tail only
           finalizes cg3.
  tail:    energies computed TRANSPOSED (eT[n,m] = k^T q) so exp writes
           attnT directly; rowsums accumulate into a [128,4] PSUM tile via
           1-wide matmuls so the softmax reciprocal is [128,4] (fast).
           Unnormalized y accumulates in 4 PSUM banks; final rescale by
           broadcast 1/rowsum.
  phase 3: the last 2 streamed chunks stay SBUF-resident -> add y in
           place and store.  The other 62 chunks stream through the same
           pool; loads chase the phase-1 stream on the sync ring and
           prefetch into ~14 free slots while the attention tail runs.
           DVE adds, scalar-ring stores.
"""

import os
import numpy as np

B, C, H, W = 4, 512, 256, 256
DS = 8
HL = H // 2            # 128 rows per core
WP = W // DS           # 32 pooled cols
NLOC = 512             # local tokens
N = 2 * NLOC           # 1024 tokens
K = C // 8             # 64
CG = C // 128          # 4 channel groups
IB = 16                # 8-row chunks per channel group
CH = CG * IB           # 64 chunks
POOL_BUFS = 16
RESIDENT = 2

_CACHE = {}
TRACE = bool(int(os.environ.get("BAM_TRACE", "0")))
LAST_EXEC_NS = None
LAST_RESULT = None


def _build():
    import concourse.bass as bass
    import concourse.tile as tile
    from concourse import bacc, mybir
    from concourse.masks import make_identity

    f32 = mybir.dt.float32
    bf16 = mybir.dt.bfloat16
    ADD = mybir.AluOpType.add
    SUB = mybir.AluOpType.subtract
    MUL = mybir.AluOpType.mult
    AXY = mybir.AxisListType.XY
    Exp = mybir.ActivationFunctionType.Exp
    Copy = mybir.ActivationFunctionType.Copy
    PAIRS = [[0, 1], [2, 3], [4, 5], [6, 7]]
    WSCALE = 1.0 / (DS * DS)

    nc = bacc.Bacc("TRN2", target_bir_lowering=False, debug=False, num_devices=8)

    x_ext = nc.dram_tensor("x", [C, HL, W], f32, kind="ExternalInput")
    wq_ext = nc.dram_tensor("wq", [K, C], f32, kind="ExternalInput")
    bq_ext = nc.dram_tensor("bq", [1, K], f32, kind="ExternalInput")
    wk_ext = nc.dram_tensor("wk", [K, C], f32, kind="ExternalInput")
    bk_ext = nc.dram_tensor("bk", [1, K], f32, kind="ExternalInput")
    wv_ext = nc.dram_tensor("wv", [C, C], f32, kind="ExternalInput")
    bv_ext = nc.dram_tensor("bv", [1, C], f32, kind="ExternalInput")
    out_ext = nc.dram_tensor("out", [C, HL, W], f32, kind="ExternalOutput")

    with tile.TileContext(nc) as tc:
        with tc.tile_pool(name="persist", bufs=1) as persist, \
             tc.tile_pool(name="scratch", bufs=2) as scratch, \
             tc.tile_pool(name="xs", bufs=POOL_BUFS) as xs, \
             tc.tile_pool(name="psW", bufs=3, space="PSUM") as psW, \
             tc.tile_pool(name="psY", bufs=1, space="PSUM") as psY, \
             tc.tile_pool(name="dram", bufs=1, space="DRAM") as dram:

            # ---- constants ----
            ident = persist.tile([128, 128], bf16, tag="ident")
            make_identity(nc, ident[:])
            identf = persist.tile([128, 128], f32, tag="identf")
            make_identity(nc, identf[:])
            ones_r = persist.tile([1, 128], bf16, tag="ones_r")
            nc.vector.memset(ones_r[:], 1.0)
            ones_rf = persist.tile([1, 128], f32, tag="ones_rf")
            nc.vector.memset(ones_rf[:], 1.0)
            ones_col = persist.tile([128, 1], bf16, tag="ones_col")
            nc.vector.memset(ones_col[:], 1.0)
            ones_n = persist.tile([1, NLOC], bf16, tag="ones_n")
            nc.vector.memset(ones_n[:], 1.0)

            def load_bias(ext, n):
                st = scratch.tile([1, n], f32, tag="bstage", name=f"bst_{ext.name}")
                nc.scalar.dma_start(out=st[:], in_=ext.ap())
                bb = persist.tile([1, n], bf16, tag=f"b_{ext.name}", name=f"b_{ext.name}")
                nc.scalar.copy(out=bb[:], in_=st[:])
                return bb

            bq_b = load_bias(bq_ext, K)
            bk_b = load_bias(bk_ext, K)
            bv_b = load_bias(bv_ext, C)

            # q/k weights: bf16, pre-scaled by 1/64, transposed per cg
            def load_qk_weight(ext):
                st = scratch.tile([K, C], f32, tag="wstage", name=f"wst_{ext.name}")
                nc.scalar.dma_start(out=st[:], in_=ext.ap())
                wb = scratch.tile([K, C], bf16, tag="wbstage", name=f"wb_{ext.name}")
                nc.scalar.activation(out=wb[:], in_=st[:], func=Copy, scale=WSCALE)
                wT = []
                for cg in range(CG):
                    ps = psW.tile([128, K], bf16, tag="w", name=f"wps_{ext.name}{cg}")
                    nc.tensor.transpose(ps[:], wb[:, cg * 128:(cg + 1) * 128],
                                        ident[0:K, 0:K])
                    t = persist.tile([128, K], bf16, tag=f"wT_{ext.name}{cg}",
                                     name=f"wT_{ext.name}{cg}")
                    nc.scalar.copy(out=t[:], in_=ps[:])
                    wT.append(t)
                return wT

            wqT = load_qk_weight(wq_ext)
            wkT = load_qk_weight(wk_ext)

            # wvT[cg][c_loc, d] = Wv[d, cg*128 + c_loc] / 64
            wvT = [persist.tile([128, C], bf16, tag=f"wvT{cg}", name=f"wvT{cg}")
                   for cg in range(CG)]
            for dt in range(CG):
                st = scratch.tile([128, C], f32, tag="wstage", name=f"wvst{dt}")
                nc.scalar.dma_start(out=st[:], in_=wv_ext.ap()[dt * 128:(dt + 1) * 128, :])
                wvb = scratch.tile([128, C], bf16, tag="wbstage", name=f"wvb{dt}")
                nc.scalar.activation(out=wvb[:], in_=st[:], func=Copy, scale=WSCALE)
                for cg in range(CG):
                    ps = psW.tile([128, 128], bf16, tag="w", name=f"wvps{dt}{cg}")
                    nc.tensor.transpose(ps[:], wvb[:, cg * 128:(cg + 1) * 128], ident[:])
                    nc.scalar.copy(out=wvT[cg][:, dt * 128:(dt + 1) * 128], in_=ps[:])

            # ---- persistent attention state (bf16 accumulators) ----
            xfb_loc = [persist.tile([128, NLOC], bf16, tag=f"xfl{cg}", name=f"xfl{cg}")
                       for cg in range(CG)]
            xfb_rem = [persist.tile([128, NLOC], bf16, tag=f"xfr{cg}", name=f"xfr{cg}")
                       for cg in range(CG)]
            vloc_acc = [persist.tile([128, C], bf16, tag=f"vla{nt}", name=f"vla{nt}")
                        for nt in range(4)]
            vrem_acc = [persist.tile([128, C], bf16, tag=f"vra{nt}", name=f"vra{nt}")
                        for nt in range(4)]
            kr_acc = persist.tile([K, NLOC], bf16, tag="kr_acc")
            # init accumulators with broadcast biases
            for nt in range(4):
                ps = psW.tile([128, C], f32, tag="w", name=f"vbias{nt}")
                nc.tensor.matmul(ps[:], ones_r[:], bv_b[:], start=True, stop=True)
                nc.vector.tensor_copy(out=vloc_acc[nt][:], in_=ps[:])
                nc.vector.tensor_copy(out=vrem_acc[nt][:], in_=ps[:])
            ps = psW.tile([K, NLOC], f32, tag="w", name="kbias")
            nc.tensor.matmul(ps[:], bk_b[:], ones_n[:], start=True, stop=True)
            nc.vector.tensor_copy(out=kr_acc[:], in_=ps[:])

            vT = [persist.tile([128, C], bf16, tag=f"vT{nt}", name=f"vT{nt}")
                  for nt in range(8)]
            attnT = [persist.tile([128, NLOC], bf16, tag=f"attnT{nt}", name=f"attnT{nt}")
                     for nt in range(8)]
            q_sb = persist.tile([K, NLOC], bf16, tag="q_sb")
            k_loc = persist.tile([K, NLOC], bf16, tag="k_loc")
            k_rem = persist.tile([K, NLOC], bf16, tag="k_rem")
            y = [persist.tile([128, NLOC], f32, tag=f"y{dt}", name=f"y{dt}")
                 for dt in range(CG)]

            # DRAM bounce buffers: pieces cg0, cg1, cg2 (512 cols), cg3 halves
            xfl_d = [dram.tile([128, NLOC], bf16, tag=f"xfl_d{cg}", name=f"xfl_d{cg}")
                     for cg in range(3)]
            xfa_d = [dram.tile([128, NLOC], bf16, tag=f"xfa_d{cg}", name=f"xfa_d{cg}")
                     for cg in range(3)]
            xfl3_d = [dram.tile([128, NLOC], bf16, tag="xfl3_d0", name="xfl3_d0")]
            xfa3_d = [dram.tile([128, NLOC], bf16, tag="xfa3_d0", name="xfa3_d0")]

            # PSUM accumulators that live across the whole attention
            y_ps = [psY.tile([128, NLOC], f32, tag=f"yps{dt}", name=f"yps{dt}")
                    for dt in range(CG)]
            rsT_ps = psY.tile([128, 4], f32, tag="rsT", name="rsT_ps")

            def stage_and_trigger(src_f32, dst_d, out_d):
                # gpsimd DMA casts f32 sums -> bf16 staging, then triggers a
                # pairwise AllReduce(add): the CC cores deliver h0+h1 directly
                nc.gpsimd.dma_start(out=dst_d[:], in_=src_f32)
                nc.gpsimd.collective_compute(
                    "AllReduce", ADD,
                    ins=[dst_d.opt()], outs=[out_d.opt()],
                    replica_groups=PAIRS)

            def recover_gpsimd(cg, xfa):
                # xfb_rem[cg] = allreduce(h0+h1) - local
                xfg = scratch.tile([128, NLOC], bf16, tag="xfg",
                                   name=f"xfg_{cg}")
                nc.gpsimd.dma_start(out=xfg[:], in_=xfa[:])
                nc.gpsimd.tensor_tensor(out=xfb_rem[cg][:], in0=xfg[:],
                                        in1=xfb_loc[cg][:], op=SUB)

            def rem_accum(cg):
                # kr/v partials for a fully-recovered remote cg; PE matmul
                # paired immediately with its DVE accumulate so PSUM slots
                # recycle fast.  Call only at stream positions where
                # xfb_rem[cg] is provably ready.
                krp = psW.tile([K, NLOC], f32, tag="w", name=f"krp{cg}")
                nc.tensor.matmul(krp[:], wkT[cg][:], xfb_rem[cg][:],
                                 start=True, stop=True)
                nc.vector.tensor_tensor(out=kr_acc[:], in0=kr_acc[:], in1=krp[:],
                                        op=ADD)
                for nt in range(4):
                    vp = psW.tile([128, C], f32, tag="w", name=f"vpr{cg}_{nt}")
                    nc.tensor.matmul(vp[:], xfb_rem[cg][:, nt * 128:(nt + 1) * 128],
                                     wvT[cg][:], start=True, stop=True)
                    nc.vector.tensor_tensor(out=vrem_acc[nt][:], in0=vrem_acc[nt][:],
                                            in1=vp[:], op=ADD)

            # ---- stream: pool + exchange + local v partials ----
            x_tiles = {}
            xf32s = []
            for cg in range(CG):
                xf32 = scratch.tile([128, NLOC], f32, tag="xf32", name=f"xf32_{cg}")
                xf32s.append(xf32)
                for ib in range(IB):
                    c = cg * IB + ib
                    t = xs.tile([128, DS, W], f32, tag="x", name=f"x1_{c}")
                    if c >= CH - RESIDENT:
                        x_tiles[c] = t
                    nc.sync.dma_start(
                        out=t[:],
                        in_=x_ext.ap()[cg * 128:(cg + 1) * 128,
                                       ib * DS:(ib + 1) * DS, :])
                    nc.vector.tensor_reduce(
                        out=xf32[:, ib * WP:(ib + 1) * WP],
                        in_=t[:].rearrange("p h (j z) -> p j h z", z=DS),
                        axis=AXY, op=ADD)
                    # deferred remote-cg accumulations at data-safe positions
                    if cg == 2 and ib == 0:
                        rem_accum(0)
                    if cg == 3 and ib == 0:
                        rem_accum(1)
                    if cg == 3 and ib == 12:
                        rem_accum(2)
                    if cg == 3 and ib == 7:
                        # cg2's exchange completed by now; recover it here so
                        # the gpsimd queue is clear before the cg3 trigger
                        recover_gpsimd(2, xfa_d[2])

                if cg < 3:
                    stage_and_trigger(xf32[:], xfl_d[cg], xfa_d[cg])
                    nc.vector.tensor_copy(out=xfb_loc[cg][:], in_=xf32[:])
                    if cg >= 1:
                        recover_gpsimd(cg - 1, xfa_d[cg - 1])
                else:
                    stage_and_trigger(xf32[:], xfl3_d[0], xfa3_d[0])
                    nc.vector.tensor_copy(out=xfb_loc[3][:], in_=xf32[:])

                # local v partials for this cg (no collective dependency)
                for nt in range(4):
                    vp = psW.tile([128, C], f32, tag="w", name=f"vp{cg}_{nt}")
                    nc.tensor.matmul(vp[:], xfb_loc[cg][:, nt * 128:(nt + 1) * 128],
                                     wvT[cg][:], start=True, stop=True)
                    nc.vector.tensor_tensor(out=vloc_acc[nt][:], in0=vloc_acc[nt][:],
                                            in1=vp[:], op=ADD)

            # ================= tail =================
            # q / k projections (tiny; run under the last exchange)
            q_ps = psW.tile([K, NLOC], f32, tag="w", name="q_ps")
            for cg in range(CG):
                nc.tensor.matmul(q_ps[:], wqT[cg][:], xfb_loc[cg][:],
                                 start=(cg == 0), stop=False)
            nc.tensor.matmul(q_ps[:], bq_b[:], ones_n[:], start=False, stop=True)
            nc.vector.tensor_copy(out=q_sb[:], in_=q_ps[:])

            kl_ps = psW.tile([K, NLOC], f32, tag="w", name="kl_ps")
            for cg in range(CG):
                nc.tensor.matmul(kl_ps[:], wkT[cg][:], xfb_loc[cg][:],
                                 start=(cg == 0), stop=False)
            nc.tensor.matmul(kl_ps[:], bk_b[:], ones_n[:], start=False, stop=True)
            nc.vector.tensor_copy(out=k_loc[:], in_=kl_ps[:])

            # finalize local v tiles (accumulated cg0..2 + bias + cg3 partial)
            for nt in range(4):
                vp = psW.tile([128, C], f32, tag="w", name=f"vp3_{nt}")
                nc.tensor.matmul(vp[:], xfb_loc[3][:, nt * 128:(nt + 1) * 128],
                                 wvT[3][:], start=True, stop=True)
                nc.vector.tensor_tensor(out=vT[nt][:], in0=vloc_acc[nt][:],
                                        in1=vp[:], op=ADD)

            def nt_block(nt, ksb, kcol):
                # transposed energies for token tile nt -> exp -> attnT[nt];
                # then rowsum + y contributions
                eT = psW.tile([128, NLOC], f32, tag="w", name=f"eT{nt}")
                for mt in range(4):
                    nc.tensor.matmul(eT[:, mt * 128:(mt + 1) * 128],
                                     ksb[:, kcol * 128:(kcol + 1) * 128],
                                     q_sb[:, mt * 128:(mt + 1) * 128],
                                     start=True, stop=True)
                nc.scalar.activation(out=attnT[nt][:], in_=eT[:], func=Exp,
                                     scale=K ** -0.5)
                for mt in range(4):
                    nc.tensor.matmul(rsT_ps[:, mt:mt + 1],
                                     attnT[nt][:, mt * 128:(mt + 1) * 128],
                                     ones_col[:],
                                     start=(nt == 0), stop=(nt == 7))
                for dt in range(CG):
                    nc.tensor.matmul(y_ps[dt][:], vT[nt][:, dt * 128:(dt + 1) * 128],
                                     attnT[nt][:], start=(nt == 0), stop=(nt == 7))

            for nt in range(4):
                nt_block(nt, k_loc, nt)

            # ---- remote half: whole cg3 recovered on DVE at the tail ----
            xfg3 = scratch.tile([128, NLOC], bf16, tag="xfg3", name="xfg3",
                                bufs=1)
            nc.scalar.dma_start(out=xfg3[:], in_=xfa3_d[0][:])
            nc.vector.tensor_tensor(out=xfb_rem[3][:], in0=xfg3[:],
                                    in1=xfb_loc[3][:], op=SUB)

            # k for the remote tokens: accumulated cg0..2+bias + cg3 partial
            krp = psW.tile([K, NLOC], f32, tag="w", name="krp3")
            nc.tensor.matmul(krp[:], wkT[3][:], xfb_rem[3][:], start=True, stop=True)
            nc.vector.tensor_tensor(out=k_rem[:], in0=kr_acc[:], in1=krp[:], op=ADD)

            # v tiles for the remote tokens
            for nt in range(4, 8):
                j = nt - 4
                vp = psW.tile([128, C], f32, tag="w", name=f"vpr3{nt}")
                nc.tensor.matmul(vp[:], xfb_rem[3][:, j * 128:(j + 1) * 128],
                                 wvT[3][:], start=True, stop=True)
                nc.vector.tensor_tensor(out=vT[nt][:], in0=vrem_acc[nt - 4][:],
                                        in1=vp[:], op=ADD)

            for nt in range(4, 8):
                nt_block(nt, k_rem, nt - 4)

            # ---- softmax rescale (transposed-reciprocal path) ----
            rsT_sb = persist.tile([128, 4], f32, tag="rsT_sb")
            nc.vector.tensor_copy(out=rsT_sb[:], in_=rsT_ps[:])
            rinvT = persist.tile([128, 4], f32, tag="rinvT")
            nc.vector.reciprocal(rinvT[:], rsT_sb[:])
            rrow = [persist.tile([1, 128], f32, tag=f"rrow{mt}", name=f"rrow{mt}")
                    for mt in range(4)]
            for mt in range(4):
                r1_ps = psW.tile([1, 128], f32, tag="w", name=f"r1_ps{mt}")
                nc.tensor.transpose(r1_ps[:], rinvT[:, mt:mt + 1], identf[:])
                nc.vector.tensor_copy(out=rrow[mt][:], in_=r1_ps[:])
            rb_ps = psW.tile([128, NLOC], f32, tag="w", name="rb_ps")
            for mt in range(4):
                nc.tensor.matmul(rb_ps[:, mt * 128:(mt + 1) * 128], ones_rf[:],
                                 rrow[mt][:], start=True, stop=True)
            rb_sb = persist.tile([128, NLOC], f32, tag="rb_sb")
            nc.vector.tensor_copy(out=rb_sb[:], in_=rb_ps[:])
            for dt in (3, 0, 1, 2):
                nc.vector.tensor_tensor(out=y[dt][:], in0=y_ps[dt][:], in1=rb_sb[:],
                                        op=MUL)

            # ---- phase 3: out = x + upsample8(y) ----
            def add_store(c, t):
                cg, ib = divmod(c, IB)
                xv = t[:].rearrange("p h (j z) -> p h j z", z=DS)
                yv = y[cg][:, ib * WP:(ib + 1) * WP][:, None, :, None] \
                    .broadcast_to([128, DS, WP, DS])
                nc.vector.tensor_tensor(out=xv, in0=xv, in1=yv, op=ADD)
                nc.scalar.dma_start(
                    out=out_ext.ap()[cg * 128:(cg + 1) * 128,
                                     ib * DS:(ib + 1) * DS, :],
                    in_=t[:])

            def add_store_split(c, t):
                cg, ib = divmod(c, IB)
                for rh in range(2):
                    xv = t[:, rh * 4:(rh + 1) * 4, :] \
                        .rearrange("p h (j z) -> p h j z", z=DS)
                    yv = y[cg][:, ib * WP:(ib + 1) * WP][:, None, :, None] \
                        .broadcast_to([128, 4, WP, DS])
                    nc.vector.tensor_tensor(out=xv, in0=xv, in1=yv, op=ADD)
                    nc.scalar.dma_start(
                        out=out_ext.ap()[cg * 128:(cg + 1) * 128,
                                         ib * DS + rh * 4:ib * DS + (rh + 1) * 4, :],
                        in_=t[:, rh * 4:(rh + 1) * 4, :])

            for c in range(CH - RESIDENT, CH):
                add_store(c, x_tiles[c])
            for c in range(CH - RESIDENT):
                cg, ib = divmod(c, IB)
                t = xs.tile([128, DS, W], f32, tag="x", name=f"x3_{c}")
                nc.sync.dma_start(
                    out=t[:],
                    in_=x_ext.ap()[cg * 128:(cg + 1) * 128,
                                   ib * DS:(ib + 1) * DS, :])
                if c >= CH - RESIDENT - 2:
                    add_store_split(c, t)
                else:
                    add_store(c, t)

    nc.finalize()
    return nc


def _get_nc():
    if "nc" not in _CACHE:
        _CACHE["nc"] = _build()
    return _CACHE["nc"]


def kernel(x, Wq, bq, Wk, bk, Wv, bv):
    global LAST_EXEC_NS, LAST_RESULT
    from concourse.bass_utils import run_bass_kernel_spmd

    x = np.asarray(x, dtype=np.float32)
    Wq = np.asarray(Wq, dtype=np.float32)
    bq = np.asarray(bq, dtype=np.float32).reshape(1, K)
    Wk = np.asarray(Wk, dtype=np.float32)
    bk = np.asarray(bk, dtype=np.float32).reshape(1, K)
    Wv = np.asarray(Wv, dtype=np.float32)
    bv = np.asarray(bv, dtype=np.float32).reshape(1, C)

    nc = _get_nc()
    in_maps = []
    for core in range(8):
        b, half = core // 2, core % 2
        in_maps.append({
            "x": np.ascontiguousarray(x[b, :, half * HL:(half + 1) * HL, :]),
            "wq": Wq, "bq": bq, "wk": Wk, "bk": bk, "wv": Wv, "bv": bv,
        })

    res = run_bass_kernel_spmd(nc, in_maps, core_ids=list(range(8)), trace=TRACE)
    LAST_EXEC_NS = res.exec_time_ns
    LAST_RESULT = res

    out = np.empty((B, C, H, W), dtype=np.float32)
    for core in range(8):
        b, half = core // 2, core % 2
        out[b, :, half * HL:(half + 1) * HL, :] = res.results[core]["out"]
    return out
